# revision 9
# baseline (speedup 1.0000x reference)
"""BasesDecomposition GNN message passing on 8 Trainium2 NeuronCores. v6.

Math (reference):
    seg  = edge_type * N + target
    h    = segment_sum(x[source] * ew, seg)        # (R, N, D)
    out  = einsum('rb,bio,rni->no', bw, bases, h)  # (N, D)

Restructured with per-edge basis coefficients c_e[b] = bw[edge_type_e, b] * ew_e:
    g[b, n, i] = sum_{e: tgt_e = n} c_e[b] * x[src_e, i]
    out[n, o]  = sum_b sum_i g[b, n, i] * bases[b, i, o]

Design:
  - Nodes sharded by target range across 8 cores (no collective).
  - Per core, nodes are grouped into NT adaptive "node tiles": contiguous
    node ranges of <= M=32 nodes, cut so that each tile has <= 512 edges of
    each source-parity.  Every tile gets a fixed 1024 edge slots
    (4x128 even + 4x128 odd) -> only ~6.5% padding.
  - The selector tile s4[slot, b*32+m] = c_e[b] * onehot(m = tgt - base) is
    built ON THE HOST (dense bf16, partition-major) and streamed with fast
    HWDGE DMA.  No per-edge device elementwise work at all.
  - x is cast to bf16 on the host and split into even/odd row tables so
    dma_gather int16 indices (src >> 1) cover N=50000 rows.
  - Per edge tile: one PE matmul  pg[i, (b,m)] += xg[e,i]^T @ s4[e,(b,m)].
  - Per node tile: ACT (scalar engine) copies pg -> gsb (bf16) arranged
    [i, (b, k, m)] for groups of 4 node tiles.
  - Per group: 4 PE matmuls  po[(k,m), o] += gsb[i, b-block]^T @ bases[b],
    copy to SBUF, one DMA to a [NT*32, D] f32 scratch output.  The host
    re-permutes scratch rows to node order (adaptive ranges differ per core).
"""

import numpy as np
import ml_dtypes

import concourse.bass as bass
import concourse.mybir as mybir
import concourse.tile as tile
from concourse import bacc
from concourse.bass_utils import run_bass_kernel_spmd

NCORES = 8
P = 128
D = 128
B = 4
M = 32          # max nodes per node tile
CAP = 512       # max edges of one source-parity per node tile
TPT = 8         # edge tiles per node tile (4 even + 4 odd)
GC = 2          # node-tile groups (of 4) per gather call batch
NT_FIXED = 208  # node tiles per core (>= greedy max over cores; mult of 4*GC)

QROT = True        # rotate SWDGE queue per gather call (paired with lane%8)
SPKT = True         # single_packet for gathers
NSWQ = 4 if QROT else 1

TRACE = False
LAST_PROFILE = None
_PROG_CACHE = {}


def _build_program(N, NT):
    fp = mybir.dt.float32
    bf = mybir.dt.bfloat16
    i16 = mybir.dt.int16
    NG = NT // 4          # groups of 4 node tiles
    NB = NG // GC         # gather batches
    NIDX = GC * 2048      # idxs per gather call (GC groups x 4 tiles x 512)

    nc = bacc.Bacc("TRN2", target_bir_lowering=False, debug=False,
                   num_devices=NCORES, num_swdge_queues=NSWQ)
    xe_d = nc.dram_tensor("xe", [N // 2, D], bf, kind="ExternalInput").ap()
    xo_d = nc.dram_tensor("xo", [N - N // 2, D], bf, kind="ExternalInput").ap()
    basesT_d = nc.dram_tensor("basesT", [P, B * D], bf, kind="ExternalInput").ap()
    # partition-major selector meta: [p][nt][t][c]
    s4_d = nc.dram_tensor("s4", [P, NT * TPT * P], bf, kind="ExternalInput").ap()
    # wrapped gather indices: [p][bat][h][GC*128 cols]
    idx_d = nc.dram_tensor("idx16", [P, NB * 2 * GC * 128], i16,
                           kind="ExternalInput").ap()
    out_d = nc.dram_tensor("out", [NT * M, D], fp, kind="ExternalOutput").ap()

    with tile.TileContext(nc) as tc:
        with (
            tc.tile_pool(name="const", bufs=1) as constp,
            tc.tile_pool(name="idxp", bufs=3) as idxp,
            tc.tile_pool(name="s4p", bufs=4) as s4p,
            tc.tile_pool(name="xgp", bufs=4) as xgp,
            tc.tile_pool(name="gsbp", bufs=2) as gsbp,
            tc.tile_pool(name="osbp", bufs=2) as osbp,
            tc.tile_pool(name="pgp", bufs=6, space="PSUM") as pgp,
            tc.tile_pool(name="pop", bufs=2, space="PSUM") as pop,
        ):
            basesT = constp.tile([P, B * D], bf)
            nc.sync.dma_start(out=basesT[:], in_=basesT_d[:])

            self_gc = [0]  # global gather-call counter (queue/lane pairing)
            # warm up the 4 SWDGE queues (first-use init off the critical path)
            if QROT:
                widx = constp.tile([P, 8], i16)
                nc.gpsimd.memset(widx[:], 0)
                for wq in range(4):
                    wout = constp.tile([P, D], bf)
                    nc.gpsimd.dma_gather(
                        out_ap=wout[:].rearrange("p (t f) -> p t f", f=D),
                        in_ap=xe_d[:],
                        idxs_ap=widx[:],
                        num_idxs=128,
                        num_idxs_reg=128,
                        elem_size=D,
                        queue_num=wq,
                    )
                    self_gc[0] += 1
            for bat in range(NB):
                # gather indices for this batch
                idxt = idxp.tile([P, 2 * GC * 128], i16, tag="idx")
                nc.sync.dma_start(
                    out=idxt[:],
                    in_=idx_d[:, bat * 2 * GC * 128:(bat + 1) * 2 * GC * 128],
                )
                # gathers in 1024-idx calls (HW ucode cap per dma_gather)
                NCALL = NIDX // 1024
                xge = xgp.tile([P, GC * 16 * D], bf, tag="xge")
                xgo = xgp.tile([P, GC * 16 * D], bf, tag="xgo")
                for half, (xg, x_t, coff) in enumerate(
                    [(xge, xe_d, 0), (xgo, xo_d, GC * 128)]
                ):
                    for j in range(NCALL):
                        q = (self_gc[0] % 8) % 4 if QROT else 0
                        self_gc[0] += 1
                        nc.gpsimd.dma_gather(
                            out_ap=xg[:, j * 8 * D:(j + 1) * 8 * D].rearrange(
                                "p (t f) -> p t f", f=D),
                            in_ap=x_t[:],
                            idxs_ap=idxt[:, coff + j * 64:coff + (j + 1) * 64],
                            num_idxs=1024,
                            num_idxs_reg=1024,
                            elem_size=D,
                            single_packet=SPKT,
                            queue_num=q,
                        )
                for gl in range(GC):
                    g = bat * GC + gl
                    s4t = s4p.tile([P, 4 * TPT * P], bf, tag="s4")
                    nc.sync.dma_start(
                        out=s4t[:],
                        in_=s4_d[:, g * 4 * TPT * P:(g + 1) * 4 * TPT * P],
                    )
                    gsb = gsbp.tile([P, B * 4 * M], bf, tag="gsb")
                    for k in range(4):
                        pg = pgp.tile([P, B * M], fp)
                        for t in range(TPT):
                            if t < 4:
                                tt = gl * 16 + k * 4 + t
                                lhsT = xge[:, tt * D:(tt + 1) * D]
                            else:
                                tt = gl * 16 + k * 4 + (t - 4)
                                lhsT = xgo[:, tt * D:(tt + 1) * D]
                            nc.tensor.matmul(
                                out=pg[:],
                                lhsT=lhsT,
                                rhs=s4t[:, (k * TPT + t) * P:(k * TPT + t + 1) * P],
                                start=(t == 0),
                                stop=(t == TPT - 1),
                            )
                        # pg[i, (b, m)] -> gsb[i, (b, k, m)]
                        nc.scalar.copy(
                            out=gsb[:].rearrange("p (b k m) -> p b k m", b=B, k=4)[
                                :, :, k, :],
                            in_=pg[:].rearrange("p (b m) -> p b m", b=B),
                        )
                    po = pop.tile([P, D], fp)
                    for b in range(B):
                        nc.tensor.matmul(
                            out=po[:],
                            lhsT=gsb[:, b * 4 * M:(b + 1) * 4 * M],
                            rhs=basesT[:, b * D:(b + 1) * D],
                            start=(b == 0),
                            stop=(b == B - 1),
                        )
                    osb = osbp.tile([P, D], fp, tag="osb")
                    nc.vector.tensor_copy(out=osb[:], in_=po[:])
                    nc.sync.dma_start(
                        out=out_d[g * P:(g + 1) * P, :], in_=osb[:]
                    )
    nc.compile()
    return nc


def _host_prep(x, src, tgt, et, ew, bw, bs):
    N, _ = x.shape
    E = src.shape[0]
    NPC = N // NCORES

    dege = np.bincount(tgt[(src & 1) == 0], minlength=N)
    dego = np.bincount(tgt[(src & 1) == 1], minlength=N)

    # greedy adaptive tiling per core
    tile_of = np.empty(N, np.int32)      # local node tile id
    base_of = np.empty(N, np.int32)      # tile base node (global id)
    max_nt = 0
    for c in range(NCORES):
        lo = c * NPC
        nt = 0
        nn = 0
        ce = 0
        co = 0
        base = lo
        for n in range(lo, lo + NPC):
            de = dege[n]
            do = dego[n]
            if nn == M or ce + de > CAP or co + do > CAP:
                nt += 1
                nn = 0
                ce = 0
                co = 0
                base = n
            tile_of[n] = nt
            base_of[n] = base
            nn += 1
            ce += de
            co += do
        max_nt = max(max_nt, nt + 1)
    # round tiles-per-core up to a multiple of one gather batch (4*GC)
    NT = max(NT_FIXED, -(-max_nt // (4 * GC)) * (4 * GC))

    core = tgt // NPC
    ntl = tile_of[tgt].astype(np.int64)      # local tile id
    h = (src & 1).astype(np.int64)
    m = (tgt - base_of[tgt]).astype(np.int64)

    gid = (core * NT + ntl) * 2 + h
    order = np.argsort(gid, kind="stable")
    gid_s = gid[order]
    counts = np.bincount(gid_s, minlength=NCORES * NT * 2)
    starts = np.zeros(NCORES * NT * 2 + 1, np.int64)
    np.cumsum(counts, out=starts[1:])
    pos = np.empty(E, np.int64)
    pos[order] = np.arange(E) - starts[gid_s]
    assert pos.max() < CAP

    slot = h * CAP + pos                      # slot within node tile [0, 1024)
    t = slot // P
    p = slot % P

    # selector meta, partition-major [NC][p][nt][t][col], col = b*M + m
    c_eb = (ew[:, None] * bw[et]).astype(ml_dtypes.bfloat16)   # (E, B)
    s4 = np.zeros((NCORES, P, NT, TPT, B * M), ml_dtypes.bfloat16)
    for b in range(B):
        s4[core, p, ntl, t, b * M + m] = c_eb[:, b]
    s4 = s4.reshape(NCORES, P, NT * TPT * B * M)

    # gather indices, wrapped: [NC][p][bat][h][j//16], j = tt*128 + p_slot
    NG = NT // 4
    NB = NG // GC
    NIDX = GC * 2048
    g = ntl // 4
    k = ntl % 4
    bat = g // GC
    gl = g % GC
    th = pos // P                              # 0..3 within parity half
    tt = gl * 16 + k * 4 + th
    j = tt * P + (pos % P)
    idxv = (src >> 1).astype(np.int16)
    idx_flat = np.zeros((NCORES, NB, 2, NIDX), np.int16)
    idx_flat[core, bat, h, j] = idxv
    # wrap16: j lives at [row j%16, col j//16], rows replicated x8
    iw = idx_flat.reshape(NCORES, NB, 2, NIDX // 16, 16)
    iw = np.swapaxes(iw, -1, -2)               # [..., 16, NIDX//16]
    iw = np.tile(iw, (1, 1, 1, 8, 1))          # [..., 128, NIDX//16]
    idx16 = np.ascontiguousarray(
        np.moveaxis(iw, 3, 1)                  # [NC, 128, NB, 2, NIDX//16]
    ).reshape(NCORES, P, NB * 2 * (NIDX // 16))

    # x tables (host cast + parity split)
    xb = x.astype(ml_dtypes.bfloat16)
    xe = np.ascontiguousarray(xb[0::2])
    xo = np.ascontiguousarray(xb[1::2])

    basesT = np.ascontiguousarray(
        bs.transpose(1, 0, 2).reshape(D, B * D)
    ).astype(ml_dtypes.bfloat16)

    # output permutation: node n (local) -> scratch row ntl*32 + (n - base)
    nodes = np.arange(N)
    rowmap = (tile_of[nodes].astype(np.int64) * M
              + nodes - base_of[nodes]).reshape(NCORES, NPC)
    return xe, xo, basesT, s4, idx16, rowmap, NT


def kernel(x, source, target, edge_type, edge_weights, base_weights, bases):
    global LAST_PROFILE
    x = np.ascontiguousarray(np.asarray(x), dtype=np.float32)
    src = np.asarray(source).astype(np.int64)
    tgt = np.asarray(target).astype(np.int64)
    et = np.asarray(edge_type).astype(np.int64)
    ew = np.ascontiguousarray(np.asarray(edge_weights), dtype=np.float32)
    bw = np.ascontiguousarray(np.asarray(base_weights), dtype=np.float32)
    bs = np.ascontiguousarray(np.asarray(bases), dtype=np.float32)

    N = x.shape[0]
    NPC = N // NCORES

    xe, xo, basesT, s4, idx16, rowmap, NT = _host_prep(x, src, tgt, et, ew, bw, bs)

    key = (N, NT)
    if key not in _PROG_CACHE:
        _PROG_CACHE[key] = _build_program(*key)
    nc = _PROG_CACHE[key]

    in_maps = [
        dict(xe=xe, xo=xo, basesT=basesT, s4=s4[c], idx16=idx16[c])
        for c in range(NCORES)
    ]
    res = run_bass_kernel_spmd(nc, in_maps, list(range(NCORES)), trace=TRACE)
    LAST_PROFILE = res
    out = np.empty((N, D), np.float32)
    for c in range(NCORES):
        scratch = res.results[c]["out"]
        out[c * NPC:(c + 1) * NPC] = scratch[rowmap[c]]
    return out


# revision 10
# speedup vs baseline: 1.0075x; 1.0075x over previous
"""BasesDecomposition GNN message passing on 8 Trainium2 NeuronCores. v6.

Math (reference):
    seg  = edge_type * N + target
    h    = segment_sum(x[source] * ew, seg)        # (R, N, D)
    out  = einsum('rb,bio,rni->no', bw, bases, h)  # (N, D)

Restructured with per-edge basis coefficients c_e[b] = bw[edge_type_e, b] * ew_e:
    g[b, n, i] = sum_{e: tgt_e = n} c_e[b] * x[src_e, i]
    out[n, o]  = sum_b sum_i g[b, n, i] * bases[b, i, o]

Design:
  - Nodes sharded by target range across 8 cores (no collective).
  - Per core, nodes are grouped into NT adaptive "node tiles": contiguous
    node ranges of <= M=32 nodes, cut so that each tile has <= 512 edges of
    each source-parity.  Every tile gets a fixed 1024 edge slots
    (4x128 even + 4x128 odd) -> only ~6.5% padding.
  - The selector tile s4[slot, b*32+m] = c_e[b] * onehot(m = tgt - base) is
    built ON THE HOST (dense bf16, partition-major) and streamed with fast
    HWDGE DMA.  No per-edge device elementwise work at all.
  - x is cast to bf16 on the host and split into even/odd row tables so
    dma_gather int16 indices (src >> 1) cover N=50000 rows.
  - Gathers are issued as 1024-idx calls (HW ucode cap) round-robined over
    the 4 SWDGE queues (queue = lane%4 pairing for Tile's DMASW sems), with
    one dummy call per queue up front to absorb first-use init.  Four calls
    run concurrently (~10.4us service each) -> ~2.6us/call sustained; the
    gather stream is the kernel's critical path.
  - Per edge tile: one PE matmul  pg[i, (b,m)] += xg[e,i]^T @ s4[e,(b,m)].
  - Per node tile: ACT (scalar engine) copies pg -> gsb (bf16) arranged
    [i, (b, k, m)] for groups of 4 node tiles.
  - Per group: 4 PE matmuls  po[(k,m), o] += gsb[i, b-block]^T @ bases[b],
    copy to SBUF, one DMA to a [NT*32, D] f32 scratch output.  The host
    re-permutes scratch rows to node order (adaptive ranges differ per core).
"""

import numpy as np
import ml_dtypes

import concourse.bass as bass
import concourse.mybir as mybir
import concourse.tile as tile
from concourse import bacc
from concourse.bass_utils import run_bass_kernel_spmd

NCORES = 8
P = 128
D = 128
B = 4
M = 32          # max nodes per node tile
CAP = 512       # max edges of one source-parity per node tile
TPT = 8         # edge tiles per node tile (4 even + 4 odd)
GC = 2          # node-tile groups (of 4) per gather call batch
NT_FIXED = 208  # node tiles per core (>= greedy max over cores; mult of 4*GC)

QROT = True        # rotate SWDGE queue per gather call (paired with lane%8)
SPKT = True         # single_packet for gathers
NSWQ = 4 if QROT else 1

TRACE = False
LAST_PROFILE = None
_PROG_CACHE = {}


def _build_program(N, NT):
    fp = mybir.dt.float32
    bf = mybir.dt.bfloat16
    i16 = mybir.dt.int16
    NG = NT // 4          # groups of 4 node tiles
    NB = NG // GC         # gather batches
    NIDX = GC * 2048      # idxs per gather call (GC groups x 4 tiles x 512)

    nc = bacc.Bacc("TRN2", target_bir_lowering=False, debug=False,
                   num_devices=NCORES, num_swdge_queues=NSWQ)
    xe_d = nc.dram_tensor("xe", [N // 2, D], bf, kind="ExternalInput").ap()
    xo_d = nc.dram_tensor("xo", [N - N // 2, D], bf, kind="ExternalInput").ap()
    basesT_d = nc.dram_tensor("basesT", [P, B * D], bf, kind="ExternalInput").ap()
    # partition-major selector meta: [p][nt][t][c]
    s4_d = nc.dram_tensor("s4", [P, NT * TPT * P], bf, kind="ExternalInput").ap()
    # wrapped gather indices: [p][bat][h][GC*128 cols]
    idx_d = nc.dram_tensor("idx16", [P, NB * 2 * GC * 128], i16,
                           kind="ExternalInput").ap()
    out_d = nc.dram_tensor("out", [NT * M, D], fp, kind="ExternalOutput").ap()

    with tile.TileContext(nc) as tc:
        with (
            tc.tile_pool(name="const", bufs=1) as constp,
            tc.tile_pool(name="idxp", bufs=3) as idxp,
            tc.tile_pool(name="s4p", bufs=4) as s4p,
            tc.tile_pool(name="xgp", bufs=4) as xgp,
            tc.tile_pool(name="gsbp", bufs=2) as gsbp,
            tc.tile_pool(name="osbp", bufs=2) as osbp,
            tc.tile_pool(name="pgp", bufs=6, space="PSUM") as pgp,
            tc.tile_pool(name="pop", bufs=2, space="PSUM") as pop,
        ):
            basesT = constp.tile([P, B * D], bf)
            nc.sync.dma_start(out=basesT[:], in_=basesT_d[:])

            self_gc = [0]  # global gather-call counter (queue/lane pairing)
            # warm up the 4 SWDGE queues (first-use init off the critical path)
            if QROT:
                widx = constp.tile([P, 8], i16)
                nc.gpsimd.memset(widx[:], 0)
                for wq in range(4):
                    wout = constp.tile([P, D], bf)
                    nc.gpsimd.dma_gather(
                        out_ap=wout[:].rearrange("p (t f) -> p t f", f=D),
                        in_ap=xe_d[:],
                        idxs_ap=widx[:],
                        num_idxs=128,
                        num_idxs_reg=128,
                        elem_size=D,
                        queue_num=wq,
                    )
                    self_gc[0] += 1
            for bat in range(NB):
                # gather indices for this batch
                idxt = idxp.tile([P, 2 * GC * 128], i16, tag="idx")
                nc.sync.dma_start(
                    out=idxt[:],
                    in_=idx_d[:, bat * 2 * GC * 128:(bat + 1) * 2 * GC * 128],
                )
                # gathers in 1024-idx calls (HW ucode cap per dma_gather)
                NCALL = NIDX // 1024
                xge = xgp.tile([P, GC * 16 * D], bf, tag="xge")
                xgo = xgp.tile([P, GC * 16 * D], bf, tag="xgo")
                for half, (xg, x_t, coff) in enumerate(
                    [(xge, xe_d, 0), (xgo, xo_d, GC * 128)]
                ):
                    for j in range(NCALL):
                        q = (self_gc[0] % 8) % 4 if QROT else 0
                        self_gc[0] += 1
                        nc.gpsimd.dma_gather(
                            out_ap=xg[:, j * 8 * D:(j + 1) * 8 * D].rearrange(
                                "p (t f) -> p t f", f=D),
                            in_ap=x_t[:],
                            idxs_ap=idxt[:, coff + j * 64:coff + (j + 1) * 64],
                            num_idxs=1024,
                            num_idxs_reg=1024,
                            elem_size=D,
                            single_packet=SPKT,
                            queue_num=q,
                        )
                for gl in range(GC):
                    g = bat * GC + gl
                    s4t = s4p.tile([P, 4 * TPT * P], bf, tag="s4")
                    nc.sync.dma_start(
                        out=s4t[:],
                        in_=s4_d[:, g * 4 * TPT * P:(g + 1) * 4 * TPT * P],
                    )
                    gsb = gsbp.tile([P, B * 4 * M], bf, tag="gsb")
                    for k in range(4):
                        pg = pgp.tile([P, B * M], fp)
                        for t in range(TPT):
                            if t < 4:
                                tt = gl * 16 + k * 4 + t
                                lhsT = xge[:, tt * D:(tt + 1) * D]
                            else:
                                tt = gl * 16 + k * 4 + (t - 4)
                                lhsT = xgo[:, tt * D:(tt + 1) * D]
                            nc.tensor.matmul(
                                out=pg[:],
                                lhsT=lhsT,
                                rhs=s4t[:, (k * TPT + t) * P:(k * TPT + t + 1) * P],
                                start=(t == 0),
                                stop=(t == TPT - 1),
                            )
                        # pg[i, (b, m)] -> gsb[i, (b, k, m)]
                        nc.scalar.copy(
                            out=gsb[:].rearrange("p (b k m) -> p b k m", b=B, k=4)[
                                :, :, k, :],
                            in_=pg[:].rearrange("p (b m) -> p b m", b=B),
                        )
                    po = pop.tile([P, D], fp)
                    for b in range(B):
                        nc.tensor.matmul(
                            out=po[:],
                            lhsT=gsb[:, b * 4 * M:(b + 1) * 4 * M],
                            rhs=basesT[:, b * D:(b + 1) * D],
                            start=(b == 0),
                            stop=(b == B - 1),
                        )
                    osb = osbp.tile([P, D], fp, tag="osb")
                    nc.vector.tensor_copy(out=osb[:], in_=po[:])
                    nc.sync.dma_start(
                        out=out_d[g * P:(g + 1) * P, :], in_=osb[:]
                    )
    nc.compile()
    return nc


def _host_prep(x, src, tgt, et, ew, bw, bs):
    N, _ = x.shape
    E = src.shape[0]
    NPC = N // NCORES

    dege = np.bincount(tgt[(src & 1) == 0], minlength=N)
    dego = np.bincount(tgt[(src & 1) == 1], minlength=N)

    # greedy adaptive tiling per core
    tile_of = np.empty(N, np.int32)      # local node tile id
    base_of = np.empty(N, np.int32)      # tile base node (global id)
    max_nt = 0
    for c in range(NCORES):
        lo = c * NPC
        nt = 0
        nn = 0
        ce = 0
        co = 0
        base = lo
        for n in range(lo, lo + NPC):
            de = dege[n]
            do = dego[n]
            if nn == M or ce + de > CAP or co + do > CAP:
                nt += 1
                nn = 0
                ce = 0
                co = 0
                base = n
            tile_of[n] = nt
            base_of[n] = base
            nn += 1
            ce += de
            co += do
        max_nt = max(max_nt, nt + 1)
    # round tiles-per-core up to a multiple of one gather batch (4*GC)
    NT = max(NT_FIXED, -(-max_nt // (4 * GC)) * (4 * GC))

    core = tgt // NPC
    ntl = tile_of[tgt].astype(np.int64)      # local tile id
    h = (src & 1).astype(np.int64)
    m = (tgt - base_of[tgt]).astype(np.int64)

    gid = (core * NT + ntl) * 2 + h
    order = np.argsort(gid, kind="stable")
    gid_s = gid[order]
    counts = np.bincount(gid_s, minlength=NCORES * NT * 2)
    starts = np.zeros(NCORES * NT * 2 + 1, np.int64)
    np.cumsum(counts, out=starts[1:])
    pos = np.empty(E, np.int64)
    pos[order] = np.arange(E) - starts[gid_s]
    assert pos.max() < CAP

    slot = h * CAP + pos                      # slot within node tile [0, 1024)
    t = slot // P
    p = slot % P

    # selector meta, partition-major [NC][p][nt][t][col], col = b*M + m
    c_eb = (ew[:, None] * bw[et]).astype(ml_dtypes.bfloat16)   # (E, B)
    s4 = np.zeros((NCORES, P, NT, TPT, B * M), ml_dtypes.bfloat16)
    for b in range(B):
        s4[core, p, ntl, t, b * M + m] = c_eb[:, b]
    s4 = s4.reshape(NCORES, P, NT * TPT * B * M)

    # gather indices, wrapped: [NC][p][bat][h][j//16], j = tt*128 + p_slot
    NG = NT // 4
    NB = NG // GC
    NIDX = GC * 2048
    g = ntl // 4
    k = ntl % 4
    bat = g // GC
    gl = g % GC
    th = pos // P                              # 0..3 within parity half
    tt = gl * 16 + k * 4 + th
    j = tt * P + (pos % P)
    idxv = (src >> 1).astype(np.int16)
    idx_flat = np.zeros((NCORES, NB, 2, NIDX), np.int16)
    idx_flat[core, bat, h, j] = idxv
    # wrap16: j lives at [row j%16, col j//16], rows replicated x8
    iw = idx_flat.reshape(NCORES, NB, 2, NIDX // 16, 16)
    iw = np.swapaxes(iw, -1, -2)               # [..., 16, NIDX//16]
    iw = np.tile(iw, (1, 1, 1, 8, 1))          # [..., 128, NIDX//16]
    idx16 = np.ascontiguousarray(
        np.moveaxis(iw, 3, 1)                  # [NC, 128, NB, 2, NIDX//16]
    ).reshape(NCORES, P, NB * 2 * (NIDX // 16))

    # x tables (host cast + parity split)
    xb = x.astype(ml_dtypes.bfloat16)
    xe = np.ascontiguousarray(xb[0::2])
    xo = np.ascontiguousarray(xb[1::2])

    basesT = np.ascontiguousarray(
        bs.transpose(1, 0, 2).reshape(D, B * D)
    ).astype(ml_dtypes.bfloat16)

    # output permutation: node n (local) -> scratch row ntl*32 + (n - base)
    nodes = np.arange(N)
    rowmap = (tile_of[nodes].astype(np.int64) * M
              + nodes - base_of[nodes]).reshape(NCORES, NPC)
    return xe, xo, basesT, s4, idx16, rowmap, NT


def kernel(x, source, target, edge_type, edge_weights, base_weights, bases):
    global LAST_PROFILE
    x = np.ascontiguousarray(np.asarray(x), dtype=np.float32)
    src = np.asarray(source).astype(np.int64)
    tgt = np.asarray(target).astype(np.int64)
    et = np.asarray(edge_type).astype(np.int64)
    ew = np.ascontiguousarray(np.asarray(edge_weights), dtype=np.float32)
    bw = np.ascontiguousarray(np.asarray(base_weights), dtype=np.float32)
    bs = np.ascontiguousarray(np.asarray(bases), dtype=np.float32)

    N = x.shape[0]
    NPC = N // NCORES

    xe, xo, basesT, s4, idx16, rowmap, NT = _host_prep(x, src, tgt, et, ew, bw, bs)

    key = (N, NT)
    if key not in _PROG_CACHE:
        _PROG_CACHE[key] = _build_program(*key)
    nc = _PROG_CACHE[key]

    in_maps = [
        dict(xe=xe, xo=xo, basesT=basesT, s4=s4[c], idx16=idx16[c])
        for c in range(NCORES)
    ]
    res = run_bass_kernel_spmd(nc, in_maps, list(range(NCORES)), trace=TRACE)
    LAST_PROFILE = res
    out = np.empty((N, D), np.float32)
    for c in range(NCORES):
        scratch = res.results[c]["out"]
        out[c * NPC:(c + 1) * NPC] = scratch[rowmap[c]]
    return out


# revision 11
# speedup vs baseline: 1.0263x; 1.0186x over previous
"""BasesDecomposition GNN message passing on 8 Trainium2 NeuronCores. v6.

Math (reference):
    seg  = edge_type * N + target
    h    = segment_sum(x[source] * ew, seg)        # (R, N, D)
    out  = einsum('rb,bio,rni->no', bw, bases, h)  # (N, D)

Restructured with per-edge basis coefficients c_e[b] = bw[edge_type_e, b] * ew_e:
    g[b, n, i] = sum_{e: tgt_e = n} c_e[b] * x[src_e, i]
    out[n, o]  = sum_b sum_i g[b, n, i] * bases[b, i, o]

Design:
  - Nodes sharded by target range across 8 cores (no collective).
  - Per core, nodes are grouped into NT adaptive "node tiles": contiguous
    node ranges of <= M=32 nodes, cut so that each tile has <= 512 edges of
    each source-parity.  Every tile gets a fixed 1024 edge slots
    (4x128 even + 4x128 odd) -> only ~6.5% padding.
  - The selector tile s4[slot, b*32+m] = c_e[b] * onehot(m = tgt - base) is
    built ON THE HOST (dense bf16, partition-major) and streamed with fast
    HWDGE DMA.  No per-edge device elementwise work at all.
  - x is cast to bf16 on the host and split into even/odd row tables so
    dma_gather int16 indices (src >> 1) cover N=50000 rows.
  - Gathers are issued as 1024-idx calls (HW ucode cap) round-robined over
    the 4 SWDGE queues (queue = lane%4 pairing for Tile's DMASW sems), with
    one dummy call per queue up front to absorb first-use init.  Four calls
    run concurrently (~10.4us service each) -> ~2.6us/call sustained; the
    gather stream is the kernel's critical path.
  - Per edge tile: one PE matmul  pg[i, (b,m)] += xg[e,i]^T @ s4[e,(b,m)].
  - Per node tile: ACT (scalar engine) copies pg -> gsb (bf16) arranged
    [i, (b, k, m)] for groups of 4 node tiles.
  - Per group: 4 PE matmuls  po[(k,m), o] += gsb[i, b-block]^T @ bases[b],
    copy to SBUF, one DMA to a [NT*32, D] f32 scratch output.  The host
    re-permutes scratch rows to node order (adaptive ranges differ per core).
"""

import numpy as np
import ml_dtypes

import concourse.bass as bass
import concourse.mybir as mybir
import concourse.tile as tile
from concourse import bacc
from concourse.bass_utils import run_bass_kernel_spmd

NCORES = 8
P = 128
D = 128
B = 4
M = 32          # max nodes per node tile
CAP = 512       # max edges of one source-parity per node tile
TPT = 8         # edge tiles per node tile (4 even + 4 odd)
GC = 2          # node-tile groups (of 4) per gather call batch
NT_FIXED = 208  # node tiles per core (>= greedy max over cores; mult of 4*GC)

QROT = True        # rotate SWDGE queue per gather call (paired with lane%8)
SPKT = True         # single_packet for gathers
NSWQ = 4 if QROT else 1

TRACE = False
LAST_PROFILE = None
_PROG_CACHE = {}


def _build_program(N, NT):
    fp = mybir.dt.float32
    bf = mybir.dt.bfloat16
    i16 = mybir.dt.int16
    NG = NT // 4          # groups of 4 node tiles
    NB = NG // GC         # gather batches
    NIDX = GC * 2048      # idxs per gather call (GC groups x 4 tiles x 512)

    nc = bacc.Bacc("TRN2", target_bir_lowering=False, debug=False,
                   num_devices=NCORES, num_swdge_queues=NSWQ)
    xe_d = nc.dram_tensor("xe", [N // 2, D], bf, kind="ExternalInput").ap()
    xo_d = nc.dram_tensor("xo", [N - N // 2, D], bf, kind="ExternalInput").ap()
    basesT_d = nc.dram_tensor("basesT", [P, B * D], bf, kind="ExternalInput").ap()
    # partition-major selector meta: [p][nt][t][c]
    s4_d = nc.dram_tensor("s4", [P, NT * TPT * P], bf, kind="ExternalInput").ap()
    # wrapped gather indices: [p][bat][h][GC*128 cols]
    idx_d = nc.dram_tensor("idx16", [P, NB * 2 * GC * 128], i16,
                           kind="ExternalInput").ap()
    out_d = nc.dram_tensor("out", [NT * M, D], fp, kind="ExternalOutput").ap()

    with tile.TileContext(nc) as tc:
        with (
            tc.tile_pool(name="const", bufs=1) as constp,
            tc.tile_pool(name="idxp", bufs=4) as idxp,
            tc.tile_pool(name="s4p", bufs=6) as s4p,
            tc.tile_pool(name="xgp", bufs=4) as xgp,
            tc.tile_pool(name="gsbp", bufs=2) as gsbp,
            tc.tile_pool(name="osbp", bufs=2) as osbp,
            tc.tile_pool(name="pgp", bufs=6, space="PSUM") as pgp,
            tc.tile_pool(name="pop", bufs=2, space="PSUM") as pop,
        ):
            basesT = constp.tile([P, B * D], bf)
            nc.sync.dma_start(out=basesT[:], in_=basesT_d[:])

            self_gc = [0]  # global gather-call counter (queue/lane pairing)
            # warm up the 4 SWDGE queues (first-use init off the critical path)
            if QROT:
                widx = constp.tile([P, 8], i16)
                nc.gpsimd.memset(widx[:], 0)
                for wq in range(4):
                    wout = constp.tile([P, D], bf)
                    nc.gpsimd.dma_gather(
                        out_ap=wout[:].rearrange("p (t f) -> p t f", f=D),
                        in_ap=xe_d[:],
                        idxs_ap=widx[:],
                        num_idxs=128,
                        num_idxs_reg=128,
                        elem_size=D,
                        queue_num=wq,
                    )
                    self_gc[0] += 1
            for bat in range(NB):
                # gather indices for this batch
                idxt = idxp.tile([P, 2 * GC * 128], i16, tag="idx")
                nc.sync.dma_start(
                    out=idxt[:],
                    in_=idx_d[:, bat * 2 * GC * 128:(bat + 1) * 2 * GC * 128],
                )
                # gathers in 1024-idx calls (HW ucode cap per dma_gather)
                NCALL = NIDX // 1024
                xge = xgp.tile([P, GC * 16 * D], bf, tag="xge")
                xgo = xgp.tile([P, GC * 16 * D], bf, tag="xgo")
                for half, (xg, x_t, coff) in enumerate(
                    [(xge, xe_d, 0), (xgo, xo_d, GC * 128)]
                ):
                    for j in range(NCALL):
                        q = (self_gc[0] % 8) % 4 if QROT else 0
                        self_gc[0] += 1
                        nc.gpsimd.dma_gather(
                            out_ap=xg[:, j * 8 * D:(j + 1) * 8 * D].rearrange(
                                "p (t f) -> p t f", f=D),
                            in_ap=x_t[:],
                            idxs_ap=idxt[:, coff + j * 64:coff + (j + 1) * 64],
                            num_idxs=1024,
                            num_idxs_reg=1024,
                            elem_size=D,
                            single_packet=SPKT,
                            queue_num=q,
                        )
                for gl in range(GC):
                    g = bat * GC + gl
                    s4t = s4p.tile([P, 4 * TPT * P], bf, tag="s4")
                    nc.sync.dma_start(
                        out=s4t[:],
                        in_=s4_d[:, g * 4 * TPT * P:(g + 1) * 4 * TPT * P],
                    )
                    gsb = gsbp.tile([P, B * 4 * M], bf, tag="gsb")
                    for k in range(4):
                        pg = pgp.tile([P, B * M], fp)
                        for t in range(TPT):
                            if t < 4:
                                tt = gl * 16 + k * 4 + t
                                lhsT = xge[:, tt * D:(tt + 1) * D]
                            else:
                                tt = gl * 16 + k * 4 + (t - 4)
                                lhsT = xgo[:, tt * D:(tt + 1) * D]
                            nc.tensor.matmul(
                                out=pg[:],
                                lhsT=lhsT,
                                rhs=s4t[:, (k * TPT + t) * P:(k * TPT + t + 1) * P],
                                start=(t == 0),
                                stop=(t == TPT - 1),
                            )
                        # pg[i, (b, m)] -> gsb[i, (b, k, m)]
                        nc.scalar.copy(
                            out=gsb[:].rearrange("p (b k m) -> p b k m", b=B, k=4)[
                                :, :, k, :],
                            in_=pg[:].rearrange("p (b m) -> p b m", b=B),
                        )
                    po = pop.tile([P, D], fp)
                    for b in range(B):
                        nc.tensor.matmul(
                            out=po[:],
                            lhsT=gsb[:, b * 4 * M:(b + 1) * 4 * M],
                            rhs=basesT[:, b * D:(b + 1) * D],
                            start=(b == 0),
                            stop=(b == B - 1),
                        )
                    osb = osbp.tile([P, D], fp, tag="osb")
                    nc.vector.tensor_copy(out=osb[:], in_=po[:])
                    nc.sync.dma_start(
                        out=out_d[g * P:(g + 1) * P, :], in_=osb[:]
                    )
    nc.compile()
    return nc


def _host_prep(x, src, tgt, et, ew, bw, bs):
    N, _ = x.shape
    E = src.shape[0]
    NPC = N // NCORES

    dege = np.bincount(tgt[(src & 1) == 0], minlength=N)
    dego = np.bincount(tgt[(src & 1) == 1], minlength=N)

    # greedy adaptive tiling per core
    tile_of = np.empty(N, np.int32)      # local node tile id
    base_of = np.empty(N, np.int32)      # tile base node (global id)
    max_nt = 0
    for c in range(NCORES):
        lo = c * NPC
        nt = 0
        nn = 0
        ce = 0
        co = 0
        base = lo
        for n in range(lo, lo + NPC):
            de = dege[n]
            do = dego[n]
            if nn == M or ce + de > CAP or co + do > CAP:
                nt += 1
                nn = 0
                ce = 0
                co = 0
                base = n
            tile_of[n] = nt
            base_of[n] = base
            nn += 1
            ce += de
            co += do
        max_nt = max(max_nt, nt + 1)
    # round tiles-per-core up to a multiple of one gather batch (4*GC)
    NT = max(NT_FIXED, -(-max_nt // (4 * GC)) * (4 * GC))

    core = tgt // NPC
    ntl = tile_of[tgt].astype(np.int64)      # local tile id
    h = (src & 1).astype(np.int64)
    m = (tgt - base_of[tgt]).astype(np.int64)

    gid = (core * NT + ntl) * 2 + h
    order = np.argsort(gid, kind="stable")
    gid_s = gid[order]
    counts = np.bincount(gid_s, minlength=NCORES * NT * 2)
    starts = np.zeros(NCORES * NT * 2 + 1, np.int64)
    np.cumsum(counts, out=starts[1:])
    pos = np.empty(E, np.int64)
    pos[order] = np.arange(E) - starts[gid_s]
    assert pos.max() < CAP

    slot = h * CAP + pos                      # slot within node tile [0, 1024)
    t = slot // P
    p = slot % P

    # selector meta, partition-major [NC][p][nt][t][col], col = b*M + m
    c_eb = (ew[:, None] * bw[et]).astype(ml_dtypes.bfloat16)   # (E, B)
    s4 = np.zeros((NCORES, P, NT, TPT, B * M), ml_dtypes.bfloat16)
    for b in range(B):
        s4[core, p, ntl, t, b * M + m] = c_eb[:, b]
    s4 = s4.reshape(NCORES, P, NT * TPT * B * M)

    # gather indices, wrapped: [NC][p][bat][h][j//16], j = tt*128 + p_slot
    NG = NT // 4
    NB = NG // GC
    NIDX = GC * 2048
    g = ntl // 4
    k = ntl % 4
    bat = g // GC
    gl = g % GC
    th = pos // P                              # 0..3 within parity half
    tt = gl * 16 + k * 4 + th
    j = tt * P + (pos % P)
    idxv = (src >> 1).astype(np.int16)
    idx_flat = np.zeros((NCORES, NB, 2, NIDX), np.int16)
    idx_flat[core, bat, h, j] = idxv
    # wrap16: j lives at [row j%16, col j//16], rows replicated x8
    iw = idx_flat.reshape(NCORES, NB, 2, NIDX // 16, 16)
    iw = np.swapaxes(iw, -1, -2)               # [..., 16, NIDX//16]
    iw = np.tile(iw, (1, 1, 1, 8, 1))          # [..., 128, NIDX//16]
    idx16 = np.ascontiguousarray(
        np.moveaxis(iw, 3, 1)                  # [NC, 128, NB, 2, NIDX//16]
    ).reshape(NCORES, P, NB * 2 * (NIDX // 16))

    # x tables (host cast + parity split)
    xb = x.astype(ml_dtypes.bfloat16)
    xe = np.ascontiguousarray(xb[0::2])
    xo = np.ascontiguousarray(xb[1::2])

    basesT = np.ascontiguousarray(
        bs.transpose(1, 0, 2).reshape(D, B * D)
    ).astype(ml_dtypes.bfloat16)

    # output permutation: node n (local) -> scratch row ntl*32 + (n - base)
    nodes = np.arange(N)
    rowmap = (tile_of[nodes].astype(np.int64) * M
              + nodes - base_of[nodes]).reshape(NCORES, NPC)
    return xe, xo, basesT, s4, idx16, rowmap, NT


def kernel(x, source, target, edge_type, edge_weights, base_weights, bases):
    global LAST_PROFILE
    x = np.ascontiguousarray(np.asarray(x), dtype=np.float32)
    src = np.asarray(source).astype(np.int64)
    tgt = np.asarray(target).astype(np.int64)
    et = np.asarray(edge_type).astype(np.int64)
    ew = np.ascontiguousarray(np.asarray(edge_weights), dtype=np.float32)
    bw = np.ascontiguousarray(np.asarray(base_weights), dtype=np.float32)
    bs = np.ascontiguousarray(np.asarray(bases), dtype=np.float32)

    N = x.shape[0]
    NPC = N // NCORES

    xe, xo, basesT, s4, idx16, rowmap, NT = _host_prep(x, src, tgt, et, ew, bw, bs)

    key = (N, NT)
    if key not in _PROG_CACHE:
        _PROG_CACHE[key] = _build_program(*key)
    nc = _PROG_CACHE[key]

    in_maps = [
        dict(xe=xe, xo=xo, basesT=basesT, s4=s4[c], idx16=idx16[c])
        for c in range(NCORES)
    ]
    res = run_bass_kernel_spmd(nc, in_maps, list(range(NCORES)), trace=TRACE)
    LAST_PROFILE = res
    out = np.empty((N, D), np.float32)
    for c in range(NCORES):
        scratch = res.results[c]["out"]
        out[c * NPC:(c + 1) * NPC] = scratch[rowmap[c]]
    return out


# revision 12
# speedup vs baseline: 1.0398x; 1.0132x over previous
"""BasesDecomposition GNN message passing on 8 Trainium2 NeuronCores. v6.

Math (reference):
    seg  = edge_type * N + target
    h    = segment_sum(x[source] * ew, seg)        # (R, N, D)
    out  = einsum('rb,bio,rni->no', bw, bases, h)  # (N, D)

Restructured with per-edge basis coefficients c_e[b] = bw[edge_type_e, b] * ew_e:
    g[b, n, i] = sum_{e: tgt_e = n} c_e[b] * x[src_e, i]
    out[n, o]  = sum_b sum_i g[b, n, i] * bases[b, i, o]

Design:
  - Nodes sharded by target range across 8 cores (no collective).
  - Per core, nodes are grouped into NT adaptive "node tiles": contiguous
    node ranges of <= M=32 nodes, cut so that each tile has <= 512 edges of
    each source-parity.  Every tile gets a fixed 1024 edge slots
    (4x128 even + 4x128 odd) -> only ~6.5% padding.
  - The selector tile s4[slot, b*32+m] = c_e[b] * onehot(m = tgt - base) is
    built ON THE HOST (dense bf16, partition-major) and streamed with fast
    HWDGE DMA.  No per-edge device elementwise work at all.
  - x is cast to bf16 on the host and split into even/odd row tables so
    dma_gather int16 indices (src >> 1) cover N=50000 rows.
  - Gathers are issued as 1024-idx calls (HW ucode cap) round-robined over
    the 4 SWDGE queues (queue = lane%4 pairing for Tile's DMASW sems), with
    one dummy call per queue up front to absorb first-use init.  Four calls
    run concurrently (~10.4us service each) -> ~2.6us/call sustained; the
    gather stream is the kernel's critical path.
  - Per edge tile: one PE matmul  pg[i, (b,m)] += xg[e,i]^T @ s4[e,(b,m)].
  - Per node tile: ACT (scalar engine) copies pg -> gsb (bf16) arranged
    [i, (b, k, m)] for groups of 4 node tiles.
  - Per group: 4 PE matmuls  po[(k,m), o] += gsb[i, b-block]^T @ bases[b],
    copy to SBUF, one DMA to a [NT*32, D] f32 scratch output.  The host
    re-permutes scratch rows to node order (adaptive ranges differ per core).
"""

import numpy as np
import ml_dtypes

import concourse.bass as bass
import concourse.mybir as mybir
import concourse.tile as tile
from concourse import bacc
from concourse.bass_utils import run_bass_kernel_spmd

NCORES = 8
P = 128
D = 128
B = 4
M = 32          # max nodes per node tile
CAP = 512       # max edges of one source-parity per node tile
TPT = 8         # edge tiles per node tile (4 even + 4 odd)
GC = 2          # node-tile groups (of 4) per gather call batch
NT_FIXED = 208  # node tiles per core (>= greedy max over cores; mult of 4*GC)

QROT = True        # rotate SWDGE queue per gather call (paired with lane%8)
SPKT = True         # single_packet for gathers
NSWQ = 4 if QROT else 1

TRACE = False
LAST_PROFILE = None
_PROG_CACHE = {}


def _build_program(N, NT):
    fp = mybir.dt.float32
    bf = mybir.dt.bfloat16
    i16 = mybir.dt.int16
    NG = NT // 4          # groups of 4 node tiles
    NB = NG // GC         # gather batches
    NIDX = GC * 2048      # idxs per gather call (GC groups x 4 tiles x 512)

    nc = bacc.Bacc("TRN2", target_bir_lowering=False, debug=False,
                   num_devices=NCORES, num_swdge_queues=NSWQ,
                   dynamic_dma_scratch_size=32768)
    xe_d = nc.dram_tensor("xe", [N // 2, D], bf, kind="ExternalInput").ap()
    xo_d = nc.dram_tensor("xo", [N - N // 2, D], bf, kind="ExternalInput").ap()
    basesT_d = nc.dram_tensor("basesT", [P, B * D], bf, kind="ExternalInput").ap()
    # partition-major selector meta: [p][nt][t][c]
    s4_d = nc.dram_tensor("s4", [P, NT * TPT * P], bf, kind="ExternalInput").ap()
    # wrapped gather indices: [p][bat][h][GC*128 cols]
    idx_d = nc.dram_tensor("idx16", [P, NB * 2 * GC * 128], i16,
                           kind="ExternalInput").ap()
    out_d = nc.dram_tensor("out", [NT * M, D], fp, kind="ExternalOutput").ap()

    with tile.TileContext(nc) as tc:
        with (
            tc.tile_pool(name="const", bufs=1) as constp,
            tc.tile_pool(name="idxp", bufs=4) as idxp,
            tc.tile_pool(name="s4p", bufs=6) as s4p,
            tc.tile_pool(name="xgp", bufs=4) as xgp,
            tc.tile_pool(name="gsbp", bufs=2) as gsbp,
            tc.tile_pool(name="osbp", bufs=2) as osbp,
            tc.tile_pool(name="pgp", bufs=6, space="PSUM") as pgp,
            tc.tile_pool(name="pop", bufs=2, space="PSUM") as pop,
        ):
            basesT = constp.tile([P, B * D], bf)
            nc.sync.dma_start(out=basesT[:], in_=basesT_d[:])

            self_gc = [0]  # global gather-call counter (queue/lane pairing)
            # warm up the 4 SWDGE queues (first-use init off the critical path)
            if QROT:
                widx = constp.tile([P, 8], i16)
                nc.gpsimd.memset(widx[:], 0)
                for wq in range(4):
                    wout = constp.tile([P, D], bf)
                    nc.gpsimd.dma_gather(
                        out_ap=wout[:].rearrange("p (t f) -> p t f", f=D),
                        in_ap=xe_d[:],
                        idxs_ap=widx[:],
                        num_idxs=128,
                        num_idxs_reg=128,
                        elem_size=D,
                        queue_num=wq,
                    )
                    self_gc[0] += 1
            for bat in range(NB):
                # gather indices for this batch
                idxt = idxp.tile([P, 2 * GC * 128], i16, tag="idx")
                nc.sync.dma_start(
                    out=idxt[:],
                    in_=idx_d[:, bat * 2 * GC * 128:(bat + 1) * 2 * GC * 128],
                )
                # gathers in 1024-idx calls (HW ucode cap per dma_gather)
                NCALL = NIDX // 1024
                xge = xgp.tile([P, GC * 16 * D], bf, tag="xge")
                xgo = xgp.tile([P, GC * 16 * D], bf, tag="xgo")
                for half, (xg, x_t, coff) in enumerate(
                    [(xge, xe_d, 0), (xgo, xo_d, GC * 128)]
                ):
                    for j in range(NCALL):
                        q = (self_gc[0] % 8) % 4 if QROT else 0
                        self_gc[0] += 1
                        nc.gpsimd.dma_gather(
                            out_ap=xg[:, j * 8 * D:(j + 1) * 8 * D].rearrange(
                                "p (t f) -> p t f", f=D),
                            in_ap=x_t[:],
                            idxs_ap=idxt[:, coff + j * 64:coff + (j + 1) * 64],
                            num_idxs=1024,
                            num_idxs_reg=1024,
                            elem_size=D,
                            single_packet=SPKT,
                            queue_num=q,
                        )
                for gl in range(GC):
                    g = bat * GC + gl
                    s4t = s4p.tile([P, 4 * TPT * P], bf, tag="s4")
                    nc.sync.dma_start(
                        out=s4t[:],
                        in_=s4_d[:, g * 4 * TPT * P:(g + 1) * 4 * TPT * P],
                    )
                    gsb = gsbp.tile([P, B * 4 * M], bf, tag="gsb")
                    for k in range(4):
                        pg = pgp.tile([P, B * M], fp)
                        for t in range(TPT):
                            if t < 4:
                                tt = gl * 16 + k * 4 + t
                                lhsT = xge[:, tt * D:(tt + 1) * D]
                            else:
                                tt = gl * 16 + k * 4 + (t - 4)
                                lhsT = xgo[:, tt * D:(tt + 1) * D]
                            nc.tensor.matmul(
                                out=pg[:],
                                lhsT=lhsT,
                                rhs=s4t[:, (k * TPT + t) * P:(k * TPT + t + 1) * P],
                                start=(t == 0),
                                stop=(t == TPT - 1),
                            )
                        # pg[i, (b, m)] -> gsb[i, (b, k, m)]
                        nc.scalar.copy(
                            out=gsb[:].rearrange("p (b k m) -> p b k m", b=B, k=4)[
                                :, :, k, :],
                            in_=pg[:].rearrange("p (b m) -> p b m", b=B),
                        )
                    po = pop.tile([P, D], fp)
                    for b in range(B):
                        nc.tensor.matmul(
                            out=po[:],
                            lhsT=gsb[:, b * 4 * M:(b + 1) * 4 * M],
                            rhs=basesT[:, b * D:(b + 1) * D],
                            start=(b == 0),
                            stop=(b == B - 1),
                        )
                    osb = osbp.tile([P, D], fp, tag="osb")
                    nc.vector.tensor_copy(out=osb[:], in_=po[:])
                    nc.sync.dma_start(
                        out=out_d[g * P:(g + 1) * P, :], in_=osb[:]
                    )
    nc.compile()
    return nc


def _host_prep(x, src, tgt, et, ew, bw, bs):
    N, _ = x.shape
    E = src.shape[0]
    NPC = N // NCORES

    dege = np.bincount(tgt[(src & 1) == 0], minlength=N)
    dego = np.bincount(tgt[(src & 1) == 1], minlength=N)

    # greedy adaptive tiling per core
    tile_of = np.empty(N, np.int32)      # local node tile id
    base_of = np.empty(N, np.int32)      # tile base node (global id)
    max_nt = 0
    for c in range(NCORES):
        lo = c * NPC
        nt = 0
        nn = 0
        ce = 0
        co = 0
        base = lo
        for n in range(lo, lo + NPC):
            de = dege[n]
            do = dego[n]
            if nn == M or ce + de > CAP or co + do > CAP:
                nt += 1
                nn = 0
                ce = 0
                co = 0
                base = n
            tile_of[n] = nt
            base_of[n] = base
            nn += 1
            ce += de
            co += do
        max_nt = max(max_nt, nt + 1)
    # round tiles-per-core up to a multiple of one gather batch (4*GC)
    NT = max(NT_FIXED, -(-max_nt // (4 * GC)) * (4 * GC))

    core = tgt // NPC
    ntl = tile_of[tgt].astype(np.int64)      # local tile id
    h = (src & 1).astype(np.int64)
    m = (tgt - base_of[tgt]).astype(np.int64)

    gid = (core * NT + ntl) * 2 + h
    order = np.argsort(gid, kind="stable")
    gid_s = gid[order]
    counts = np.bincount(gid_s, minlength=NCORES * NT * 2)
    starts = np.zeros(NCORES * NT * 2 + 1, np.int64)
    np.cumsum(counts, out=starts[1:])
    pos = np.empty(E, np.int64)
    pos[order] = np.arange(E) - starts[gid_s]
    assert pos.max() < CAP

    slot = h * CAP + pos                      # slot within node tile [0, 1024)
    t = slot // P
    p = slot % P

    # selector meta, partition-major [NC][p][nt][t][col], col = b*M + m
    c_eb = (ew[:, None] * bw[et]).astype(ml_dtypes.bfloat16)   # (E, B)
    s4 = np.zeros((NCORES, P, NT, TPT, B * M), ml_dtypes.bfloat16)
    for b in range(B):
        s4[core, p, ntl, t, b * M + m] = c_eb[:, b]
    s4 = s4.reshape(NCORES, P, NT * TPT * B * M)

    # gather indices, wrapped: [NC][p][bat][h][j//16], j = tt*128 + p_slot
    NG = NT // 4
    NB = NG // GC
    NIDX = GC * 2048
    g = ntl // 4
    k = ntl % 4
    bat = g // GC
    gl = g % GC
    th = pos // P                              # 0..3 within parity half
    tt = gl * 16 + k * 4 + th
    j = tt * P + (pos % P)
    idxv = (src >> 1).astype(np.int16)
    idx_flat = np.zeros((NCORES, NB, 2, NIDX), np.int16)
    idx_flat[core, bat, h, j] = idxv
    # wrap16: j lives at [row j%16, col j//16], rows replicated x8
    iw = idx_flat.reshape(NCORES, NB, 2, NIDX // 16, 16)
    iw = np.swapaxes(iw, -1, -2)               # [..., 16, NIDX//16]
    iw = np.tile(iw, (1, 1, 1, 8, 1))          # [..., 128, NIDX//16]
    idx16 = np.ascontiguousarray(
        np.moveaxis(iw, 3, 1)                  # [NC, 128, NB, 2, NIDX//16]
    ).reshape(NCORES, P, NB * 2 * (NIDX // 16))

    # x tables (host cast + parity split)
    xb = x.astype(ml_dtypes.bfloat16)
    xe = np.ascontiguousarray(xb[0::2])
    xo = np.ascontiguousarray(xb[1::2])

    basesT = np.ascontiguousarray(
        bs.transpose(1, 0, 2).reshape(D, B * D)
    ).astype(ml_dtypes.bfloat16)

    # output permutation: node n (local) -> scratch row ntl*32 + (n - base)
    nodes = np.arange(N)
    rowmap = (tile_of[nodes].astype(np.int64) * M
              + nodes - base_of[nodes]).reshape(NCORES, NPC)
    return xe, xo, basesT, s4, idx16, rowmap, NT


def kernel(x, source, target, edge_type, edge_weights, base_weights, bases):
    global LAST_PROFILE
    x = np.ascontiguousarray(np.asarray(x), dtype=np.float32)
    src = np.asarray(source).astype(np.int64)
    tgt = np.asarray(target).astype(np.int64)
    et = np.asarray(edge_type).astype(np.int64)
    ew = np.ascontiguousarray(np.asarray(edge_weights), dtype=np.float32)
    bw = np.ascontiguousarray(np.asarray(base_weights), dtype=np.float32)
    bs = np.ascontiguousarray(np.asarray(bases), dtype=np.float32)

    N = x.shape[0]
    NPC = N // NCORES

    xe, xo, basesT, s4, idx16, rowmap, NT = _host_prep(x, src, tgt, et, ew, bw, bs)

    key = (N, NT)
    if key not in _PROG_CACHE:
        _PROG_CACHE[key] = _build_program(*key)
    nc = _PROG_CACHE[key]

    in_maps = [
        dict(xe=xe, xo=xo, basesT=basesT, s4=s4[c], idx16=idx16[c])
        for c in range(NCORES)
    ]
    res = run_bass_kernel_spmd(nc, in_maps, list(range(NCORES)), trace=TRACE)
    LAST_PROFILE = res
    out = np.empty((N, D), np.float32)
    for c in range(NCORES):
        scratch = res.results[c]["out"]
        out[c * NPC:(c + 1) * NPC] = scratch[rowmap[c]]
    return out


# revision 13
# speedup vs baseline: 1.2000x; 1.1540x over previous
"""BasesDecomposition GNN message passing on 8 Trainium2 NeuronCores. v6.

Math (reference):
    seg  = edge_type * N + target
    h    = segment_sum(x[source] * ew, seg)        # (R, N, D)
    out  = einsum('rb,bio,rni->no', bw, bases, h)  # (N, D)

Restructured with per-edge basis coefficients c_e[b] = bw[edge_type_e, b] * ew_e:
    g[b, n, i] = sum_{e: tgt_e = n} c_e[b] * x[src_e, i]
    out[n, o]  = sum_b sum_i g[b, n, i] * bases[b, i, o]

Design:
  - Nodes sharded by target range across 8 cores (no collective).
  - Per core, nodes are grouped into NT adaptive "node tiles": contiguous
    node ranges of <= M=32 nodes, cut so that each tile has <= 512 edges of
    each source-parity.  Every tile gets a fixed 1024 edge slots
    (4x128 even + 4x128 odd) -> only ~6.5% padding.
  - The selector tile s4[slot, b*32+m] = c_e[b] * onehot(m = tgt - base) is
    built ON THE HOST (dense bf16, partition-major) and streamed with fast
    HWDGE DMA.  No per-edge device elementwise work at all.
  - x is cast to bf16 on the host and split into even/odd row tables so
    dma_gather int16 indices (src >> 1) cover N=50000 rows.
  - Gathers are issued as 1024-idx calls (HW ucode cap) round-robined over
    the 4 SWDGE queues (queue = lane%4 pairing for Tile's DMASW sems), with
    one dummy call per queue up front to absorb first-use init.  Four calls
    run concurrently (~10.4us service each) -> ~2.6us/call sustained; the
    gather stream is the kernel's critical path.
  - Per edge tile: one PE matmul  pg[i, (b,m)] += xg[e,i]^T @ s4[e,(b,m)].
  - Per node tile: ACT (scalar engine) copies pg -> gsb (bf16) arranged
    [i, (b, k, m)] for groups of 4 node tiles.
  - Per group: 4 PE matmuls  po[(k,m), o] += gsb[i, b-block]^T @ bases[b],
    copy to SBUF, one DMA to a [NT*32, D] f32 scratch output.  The host
    re-permutes scratch rows to node order (adaptive ranges differ per core).
"""

import numpy as np
import ml_dtypes

import concourse.bass as bass
import concourse.mybir as mybir
import concourse.tile as tile
from concourse import bacc
from concourse.bass_utils import run_bass_kernel_spmd

NCORES = 8
P = 128
D = 128
B = 4
M = 32          # max nodes per node tile
CAP = 512       # max edges of one source-parity per node tile
TPT = 8         # edge tiles per node tile (4 even + 4 odd)
GC = 2          # node-tile groups (of 4) per gather call batch
NT_FIXED = 208  # node tiles per core (>= greedy max over cores; mult of 4*GC)

QROT = True        # rotate SWDGE queue per gather call (paired with lane%8)
SPKT = True         # single_packet for gathers
NSWQ = 4 if QROT else 1

TRACE = False
LAST_PROFILE = None
_PROG_CACHE = {}


def _build_program(N, NT):
    fp = mybir.dt.float32
    bf = mybir.dt.bfloat16
    i16 = mybir.dt.int16
    NG = NT // 4          # groups of 4 node tiles
    NB = NG // GC         # gather batches
    NIDX = GC * 2048      # idxs per gather call (GC groups x 4 tiles x 512)

    nc = bacc.Bacc("TRN2", target_bir_lowering=False, debug=False,
                   num_devices=NCORES, num_swdge_queues=NSWQ,
                   dynamic_dma_scratch_size=32768)
    xe_d = nc.dram_tensor("xe", [N // 2, D], bf, kind="ExternalInput").ap()
    xo_d = nc.dram_tensor("xo", [N - N // 2, D], bf, kind="ExternalInput").ap()
    basesT_d = nc.dram_tensor("basesT", [P, B * D], bf, kind="ExternalInput").ap()
    # partition-major selector meta: [p][nt][t][c]
    s4_d = nc.dram_tensor("s4", [P, NT * TPT * P], bf, kind="ExternalInput").ap()
    # wrapped gather indices: [p][bat][h][GC*128 cols]
    idx_d = nc.dram_tensor("idx16", [P, NB * 2 * GC * 128], i16,
                           kind="ExternalInput").ap()
    out_d = nc.dram_tensor("out", [NT * M, D], fp, kind="ExternalOutput").ap()

    with tile.TileContext(nc) as tc:
        with (
            tc.tile_pool(name="const", bufs=1) as constp,
            tc.tile_pool(name="idxp", bufs=4) as idxp,
            tc.tile_pool(name="s4p", bufs=6) as s4p,
            tc.tile_pool(name="xgp", bufs=4) as xgp,
            tc.tile_pool(name="gsbp", bufs=2) as gsbp,
            tc.tile_pool(name="osbp", bufs=2) as osbp,
            tc.tile_pool(name="pgp", bufs=6, space="PSUM") as pgp,
            tc.tile_pool(name="pop", bufs=2, space="PSUM") as pop,
        ):
            basesT = constp.tile([P, B * D], bf)
            nc.sync.dma_start(out=basesT[:], in_=basesT_d[:])

            self_gc = [0]  # global gather-call counter (queue/lane pairing)
            # warm up the 4 SWDGE queues (first-use init off the critical path)
            if QROT:
                widx = constp.tile([P, 8], i16)
                nc.gpsimd.memset(widx[:], 0)
                for wq in range(4):
                    wout = constp.tile([P, D], bf, tag=f"wout{wq}")
                    nc.gpsimd.dma_gather(
                        out_ap=wout[:].rearrange("p (t f) -> p t f", f=D),
                        in_ap=xe_d[:],
                        idxs_ap=widx[:],
                        num_idxs=128,
                        num_idxs_reg=128,
                        elem_size=D,
                        queue_num=wq,
                    )
                    self_gc[0] += 1
            for bat in range(NB):
                # gather indices for this batch
                idxt = idxp.tile([P, 2 * GC * 128], i16, tag="idx")
                nc.sync.dma_start(
                    out=idxt[:],
                    in_=idx_d[:, bat * 2 * GC * 128:(bat + 1) * 2 * GC * 128],
                )
                # gathers in 1024-idx calls (HW ucode cap per dma_gather)
                NCALL = NIDX // 1024
                xge = xgp.tile([P, GC * 16 * D], bf, tag="xge")
                xgo = xgp.tile([P, GC * 16 * D], bf, tag="xgo")
                for half, (xg, x_t, coff) in enumerate(
                    [(xge, xe_d, 0), (xgo, xo_d, GC * 128)]
                ):
                    for j in range(NCALL):
                        q = (self_gc[0] % 8) % 4 if QROT else 0
                        self_gc[0] += 1
                        nc.gpsimd.dma_gather(
                            out_ap=xg[:, j * 8 * D:(j + 1) * 8 * D].rearrange(
                                "p (t f) -> p t f", f=D),
                            in_ap=x_t[:],
                            idxs_ap=idxt[:, coff + j * 64:coff + (j + 1) * 64],
                            num_idxs=1024,
                            num_idxs_reg=1024,
                            elem_size=D,
                            single_packet=SPKT,
                            queue_num=q,
                        )
                for gl in range(GC):
                    g = bat * GC + gl
                    s4t = s4p.tile([P, 4 * TPT * P], bf, tag="s4")
                    nc.sync.dma_start(
                        out=s4t[:],
                        in_=s4_d[:, g * 4 * TPT * P:(g + 1) * 4 * TPT * P],
                    )
                    gsb = gsbp.tile([P, B * 4 * M], bf, tag="gsb")
                    for k in range(4):
                        pg = pgp.tile([P, B * M], fp)
                        for t in range(TPT):
                            if t < 4:
                                tt = gl * 16 + k * 4 + t
                                lhsT = xge[:, tt * D:(tt + 1) * D]
                            else:
                                tt = gl * 16 + k * 4 + (t - 4)
                                lhsT = xgo[:, tt * D:(tt + 1) * D]
                            nc.tensor.matmul(
                                out=pg[:],
                                lhsT=lhsT,
                                rhs=s4t[:, (k * TPT + t) * P:(k * TPT + t + 1) * P],
                                start=(t == 0),
                                stop=(t == TPT - 1),
                            )
                        # pg[i, (b, m)] -> gsb[i, (b, k, m)]
                        nc.scalar.copy(
                            out=gsb[:].rearrange("p (b k m) -> p b k m", b=B, k=4)[
                                :, :, k, :],
                            in_=pg[:].rearrange("p (b m) -> p b m", b=B),
                        )
                    po = pop.tile([P, D], fp)
                    for b in range(B):
                        nc.tensor.matmul(
                            out=po[:],
                            lhsT=gsb[:, b * 4 * M:(b + 1) * 4 * M],
                            rhs=basesT[:, b * D:(b + 1) * D],
                            start=(b == 0),
                            stop=(b == B - 1),
                        )
                    osb = osbp.tile([P, D], fp, tag="osb")
                    nc.vector.tensor_copy(out=osb[:], in_=po[:])
                    # scalar-engine HWDGE ring: keeps these writes (which wait
                    # on the epilogue chain) from head-of-line-blocking the s4
                    # loads queued on the sync ring
                    nc.scalar.dma_start(
                        out=out_d[g * P:(g + 1) * P, :], in_=osb[:]
                    )
    nc.compile()
    return nc


def _host_prep(x, src, tgt, et, ew, bw, bs):
    N, _ = x.shape
    E = src.shape[0]
    NPC = N // NCORES

    dege = np.bincount(tgt[(src & 1) == 0], minlength=N)
    dego = np.bincount(tgt[(src & 1) == 1], minlength=N)

    # greedy adaptive tiling per core
    tile_of = np.empty(N, np.int32)      # local node tile id
    base_of = np.empty(N, np.int32)      # tile base node (global id)
    max_nt = 0
    for c in range(NCORES):
        lo = c * NPC
        nt = 0
        nn = 0
        ce = 0
        co = 0
        base = lo
        for n in range(lo, lo + NPC):
            de = dege[n]
            do = dego[n]
            if nn == M or ce + de > CAP or co + do > CAP:
                nt += 1
                nn = 0
                ce = 0
                co = 0
                base = n
            tile_of[n] = nt
            base_of[n] = base
            nn += 1
            ce += de
            co += do
        max_nt = max(max_nt, nt + 1)
    # round tiles-per-core up to a multiple of one gather batch (4*GC)
    NT = max(NT_FIXED, -(-max_nt // (4 * GC)) * (4 * GC))

    core = tgt // NPC
    ntl = tile_of[tgt].astype(np.int64)      # local tile id
    h = (src & 1).astype(np.int64)
    m = (tgt - base_of[tgt]).astype(np.int64)

    gid = (core * NT + ntl) * 2 + h
    order = np.argsort(gid, kind="stable")
    gid_s = gid[order]
    counts = np.bincount(gid_s, minlength=NCORES * NT * 2)
    starts = np.zeros(NCORES * NT * 2 + 1, np.int64)
    np.cumsum(counts, out=starts[1:])
    pos = np.empty(E, np.int64)
    pos[order] = np.arange(E) - starts[gid_s]
    assert pos.max() < CAP

    slot = h * CAP + pos                      # slot within node tile [0, 1024)
    t = slot // P
    p = slot % P

    # selector meta, partition-major [NC][p][nt][t][col], col = b*M + m
    c_eb = (ew[:, None] * bw[et]).astype(ml_dtypes.bfloat16)   # (E, B)
    s4 = np.zeros((NCORES, P, NT, TPT, B * M), ml_dtypes.bfloat16)
    for b in range(B):
        s4[core, p, ntl, t, b * M + m] = c_eb[:, b]
    s4 = s4.reshape(NCORES, P, NT * TPT * B * M)

    # gather indices, wrapped: [NC][p][bat][h][j//16], j = tt*128 + p_slot
    NG = NT // 4
    NB = NG // GC
    NIDX = GC * 2048
    g = ntl // 4
    k = ntl % 4
    bat = g // GC
    gl = g % GC
    th = pos // P                              # 0..3 within parity half
    tt = gl * 16 + k * 4 + th
    j = tt * P + (pos % P)
    idxv = (src >> 1).astype(np.int16)
    idx_flat = np.zeros((NCORES, NB, 2, NIDX), np.int16)
    idx_flat[core, bat, h, j] = idxv
    # wrap16: j lives at [row j%16, col j//16], rows replicated x8
    iw = idx_flat.reshape(NCORES, NB, 2, NIDX // 16, 16)
    iw = np.swapaxes(iw, -1, -2)               # [..., 16, NIDX//16]
    iw = np.tile(iw, (1, 1, 1, 8, 1))          # [..., 128, NIDX//16]
    idx16 = np.ascontiguousarray(
        np.moveaxis(iw, 3, 1)                  # [NC, 128, NB, 2, NIDX//16]
    ).reshape(NCORES, P, NB * 2 * (NIDX // 16))

    # x tables (host cast + parity split)
    xb = x.astype(ml_dtypes.bfloat16)
    xe = np.ascontiguousarray(xb[0::2])
    xo = np.ascontiguousarray(xb[1::2])

    basesT = np.ascontiguousarray(
        bs.transpose(1, 0, 2).reshape(D, B * D)
    ).astype(ml_dtypes.bfloat16)

    # output permutation: node n (local) -> scratch row ntl*32 + (n - base)
    nodes = np.arange(N)
    rowmap = (tile_of[nodes].astype(np.int64) * M
              + nodes - base_of[nodes]).reshape(NCORES, NPC)
    return xe, xo, basesT, s4, idx16, rowmap, NT


def kernel(x, source, target, edge_type, edge_weights, base_weights, bases):
    global LAST_PROFILE
    x = np.ascontiguousarray(np.asarray(x), dtype=np.float32)
    src = np.asarray(source).astype(np.int64)
    tgt = np.asarray(target).astype(np.int64)
    et = np.asarray(edge_type).astype(np.int64)
    ew = np.ascontiguousarray(np.asarray(edge_weights), dtype=np.float32)
    bw = np.ascontiguousarray(np.asarray(base_weights), dtype=np.float32)
    bs = np.ascontiguousarray(np.asarray(bases), dtype=np.float32)

    N = x.shape[0]
    NPC = N // NCORES

    xe, xo, basesT, s4, idx16, rowmap, NT = _host_prep(x, src, tgt, et, ew, bw, bs)

    key = (N, NT)
    if key not in _PROG_CACHE:
        _PROG_CACHE[key] = _build_program(*key)
    nc = _PROG_CACHE[key]

    in_maps = [
        dict(xe=xe, xo=xo, basesT=basesT, s4=s4[c], idx16=idx16[c])
        for c in range(NCORES)
    ]
    res = run_bass_kernel_spmd(nc, in_maps, list(range(NCORES)), trace=TRACE)
    LAST_PROFILE = res
    out = np.empty((N, D), np.float32)
    for c in range(NCORES):
        scratch = res.results[c]["out"]
        out[c * NPC:(c + 1) * NPC] = scratch[rowmap[c]]
    return out


# revision 14
# speedup vs baseline: 1.2117x; 1.0098x over previous
"""BasesDecomposition GNN message passing on 8 Trainium2 NeuronCores. v6.

Math (reference):
    seg  = edge_type * N + target
    h    = segment_sum(x[source] * ew, seg)        # (R, N, D)
    out  = einsum('rb,bio,rni->no', bw, bases, h)  # (N, D)

Restructured with per-edge basis coefficients c_e[b] = bw[edge_type_e, b] * ew_e:
    g[b, n, i] = sum_{e: tgt_e = n} c_e[b] * x[src_e, i]
    out[n, o]  = sum_b sum_i g[b, n, i] * bases[b, i, o]

Design:
  - Nodes sharded by target range across 8 cores (no collective).
  - Per core, nodes are grouped into NT adaptive "node tiles": contiguous
    node ranges of <= M=32 nodes, cut so that each tile has <= 512 edges of
    each source-parity.  Every tile gets a fixed 1024 edge slots
    (4x128 even + 4x128 odd) -> only ~6.5% padding.
  - The selector tile s4[slot, b*32+m] = c_e[b] * onehot(m = tgt - base) is
    built ON THE HOST (dense bf16, partition-major) and streamed with fast
    HWDGE DMA.  No per-edge device elementwise work at all.
  - x is cast to bf16 on the host and split into even/odd row tables so
    dma_gather int16 indices (src >> 1) cover N=50000 rows.
  - Gathers are issued as 1024-idx calls (HW ucode cap) round-robined over
    the 4 SWDGE queues (queue = lane%4 pairing for Tile's DMASW sems), with
    one dummy call per queue up front to absorb first-use init.  Four calls
    run concurrently (~10.4us service each) -> ~2.6us/call sustained; the
    gather stream is the kernel's critical path.
  - Per edge tile: one PE matmul  pg[i, (b,m)] += xg[e,i]^T @ s4[e,(b,m)].
  - Per node tile: ACT (scalar engine) copies pg -> gsb (bf16) arranged
    [i, (b, k, m)] for groups of 4 node tiles.
  - Per group: 4 PE matmuls  po[(k,m), o] += gsb[i, b-block]^T @ bases[b],
    copy to SBUF, one DMA to a [NT*32, D] f32 scratch output.  The host
    re-permutes scratch rows to node order (adaptive ranges differ per core).
"""

import numpy as np
import ml_dtypes

import concourse.bass as bass
import concourse.mybir as mybir
import concourse.tile as tile
from concourse import bacc
from concourse.bass_utils import run_bass_kernel_spmd

NCORES = 8
P = 128
D = 128
B = 4
M = 32          # max nodes per node tile
CAP = 512       # max edges of one source-parity per node tile
TPT = 8         # edge tiles per node tile (4 even + 4 odd)
GC = 2          # node-tile groups (of 4) per gather call batch
NT_FIXED = 208  # node tiles per core (>= greedy max over cores; mult of 4*GC)

QROT = True        # rotate SWDGE queue per gather call (paired with lane%8)
SPKT = True         # single_packet for gathers
NSWQ = 4 if QROT else 1

TRACE = False
LAST_PROFILE = None
_PROG_CACHE = {}


def _build_program(N, NT):
    fp = mybir.dt.float32
    bf = mybir.dt.bfloat16
    i16 = mybir.dt.int16
    NG = NT // 4          # groups of 4 node tiles
    NB = NG // GC         # gather batches
    NIDX = GC * 2048      # idxs per gather call (GC groups x 4 tiles x 512)

    nc = bacc.Bacc("TRN2", target_bir_lowering=False, debug=False,
                   num_devices=NCORES, num_swdge_queues=NSWQ,
                   dynamic_dma_scratch_size=32768)
    xe_d = nc.dram_tensor("xe", [N // 2, D], bf, kind="ExternalInput").ap()
    xo_d = nc.dram_tensor("xo", [N - N // 2, D], bf, kind="ExternalInput").ap()
    basesT_d = nc.dram_tensor("basesT", [P, B * D], bf, kind="ExternalInput").ap()
    # partition-major selector meta: [p][nt][t][c]
    s4_d = nc.dram_tensor("s4", [P, NT * TPT * P], bf, kind="ExternalInput").ap()
    # wrapped gather indices: [p][bat][h][GC*128 cols]
    idx_d = nc.dram_tensor("idx16", [P, NB * 2 * GC * 128], i16,
                           kind="ExternalInput").ap()
    out_d = nc.dram_tensor("out", [NT * M, D], fp, kind="ExternalOutput").ap()

    with tile.TileContext(nc) as tc:
        with (
            tc.tile_pool(name="const", bufs=1) as constp,
            tc.tile_pool(name="idxp", bufs=4) as idxp,
            tc.tile_pool(name="s4p", bufs=6) as s4p,
            tc.tile_pool(name="xgp", bufs=4) as xgp,
            tc.tile_pool(name="gsbp", bufs=2) as gsbp,
            tc.tile_pool(name="osbp", bufs=2) as osbp,
            tc.tile_pool(name="pgp", bufs=6, space="PSUM") as pgp,
            tc.tile_pool(name="pop", bufs=2, space="PSUM") as pop,
        ):
            basesT = constp.tile([P, B * D], bf)
            nc.sync.dma_start(out=basesT[:], in_=basesT_d[:])

            self_gc = [0]  # global gather-call counter (queue/lane pairing)
            # warm up the 4 SWDGE queues (first-use init off the critical path)
            if QROT:
                widx = constp.tile([P, 8], i16)
                nc.gpsimd.memset(widx[:], 0)
                for wq in range(4):
                    wout = constp.tile([P, D], bf, tag=f"wout{wq}")
                    nc.gpsimd.dma_gather(
                        out_ap=wout[:].rearrange("p (t f) -> p t f", f=D),
                        in_ap=xe_d[:],
                        idxs_ap=widx[:],
                        num_idxs=128,
                        num_idxs_reg=128,
                        elem_size=D,
                        queue_num=wq,
                    )
                    self_gc[0] += 1
            for bat in range(NB):
                # gather indices for this batch
                idxt = idxp.tile([P, 2 * GC * 128], i16, tag="idx")
                nc.sync.dma_start(
                    out=idxt[:],
                    in_=idx_d[:, bat * 2 * GC * 128:(bat + 1) * 2 * GC * 128],
                )
                # gathers in 1024-idx calls (HW ucode cap per dma_gather)
                NCALL = NIDX // 1024
                xge = xgp.tile([P, GC * 16 * D], bf, tag="xge")
                xgo = xgp.tile([P, GC * 16 * D], bf, tag="xgo")
                for half, (xg, x_t, coff) in enumerate(
                    [(xge, xe_d, 0), (xgo, xo_d, GC * 128)]
                ):
                    for j in range(NCALL):
                        q = (self_gc[0] % 8) % 4 if QROT else 0
                        self_gc[0] += 1
                        nc.gpsimd.dma_gather(
                            out_ap=xg[:, j * 8 * D:(j + 1) * 8 * D].rearrange(
                                "p (t f) -> p t f", f=D),
                            in_ap=x_t[:],
                            idxs_ap=idxt[:, coff + j * 64:coff + (j + 1) * 64],
                            num_idxs=1024,
                            num_idxs_reg=1024,
                            elem_size=D,
                            single_packet=SPKT,
                            queue_num=q,
                        )
                for gl in range(GC):
                    g = bat * GC + gl
                    s4t = s4p.tile([P, 4 * TPT * P], bf, tag="s4")
                    nc.sync.dma_start(
                        out=s4t[:],
                        in_=s4_d[:, g * 4 * TPT * P:(g + 1) * 4 * TPT * P],
                    )
                    gsb = gsbp.tile([P, B * 4 * M], bf, tag="gsb")
                    for k in range(4):
                        pg = pgp.tile([P, B * M], fp)
                        for t in range(TPT):
                            if t < 4:
                                tt = gl * 16 + k * 4 + t
                                lhsT = xge[:, tt * D:(tt + 1) * D]
                            else:
                                tt = gl * 16 + k * 4 + (t - 4)
                                lhsT = xgo[:, tt * D:(tt + 1) * D]
                            nc.tensor.matmul(
                                out=pg[:],
                                lhsT=lhsT,
                                rhs=s4t[:, (k * TPT + t) * P:(k * TPT + t + 1) * P],
                                start=(t == 0),
                                stop=(t == TPT - 1),
                            )
                        # pg[i, (b, m)] -> gsb[i, (b, k, m)]
                        nc.scalar.copy(
                            out=gsb[:].rearrange("p (b k m) -> p b k m", b=B, k=4)[
                                :, :, k, :],
                            in_=pg[:].rearrange("p (b m) -> p b m", b=B),
                        )
                    po = pop.tile([P, D], fp)
                    for b in range(B):
                        nc.tensor.matmul(
                            out=po[:],
                            lhsT=gsb[:, b * 4 * M:(b + 1) * 4 * M],
                            rhs=basesT[:, b * D:(b + 1) * D],
                            start=(b == 0),
                            stop=(b == B - 1),
                        )
                    osb = osbp.tile([P, D], fp, tag="osb")
                    nc.vector.tensor_copy(out=osb[:], in_=po[:])
                    # scalar-engine HWDGE ring: keeps these writes (which wait
                    # on the epilogue chain) from head-of-line-blocking the s4
                    # loads queued on the sync ring
                    nc.scalar.dma_start(
                        out=out_d[g * P:(g + 1) * P, :], in_=osb[:]
                    )
    nc.compile()
    return nc


def _host_prep(x, src, tgt, et, ew, bw, bs):
    N, _ = x.shape
    E = src.shape[0]
    NPC = N // NCORES

    dege = np.bincount(tgt[(src & 1) == 0], minlength=N)
    dego = np.bincount(tgt[(src & 1) == 1], minlength=N)

    # greedy adaptive tiling per core
    tile_of = np.empty(N, np.int32)      # local node tile id
    base_of = np.empty(N, np.int32)      # tile base node (global id)
    max_nt = 0
    for c in range(NCORES):
        lo = c * NPC
        nt = 0
        nn = 0
        ce = 0
        co = 0
        base = lo
        for n in range(lo, lo + NPC):
            de = dege[n]
            do = dego[n]
            if nn == M or ce + de > CAP or co + do > CAP:
                nt += 1
                nn = 0
                ce = 0
                co = 0
                base = n
            tile_of[n] = nt
            base_of[n] = base
            nn += 1
            ce += de
            co += do
        max_nt = max(max_nt, nt + 1)
    # round tiles-per-core up to a multiple of one gather batch (4*GC)
    NT = max(NT_FIXED, -(-max_nt // (4 * GC)) * (4 * GC))

    core = tgt // NPC
    ntl = tile_of[tgt].astype(np.int64)      # local tile id
    h = (src & 1).astype(np.int64)
    m = (tgt - base_of[tgt]).astype(np.int64)

    gid = (core * NT + ntl) * 2 + h
    order = np.argsort(gid, kind="stable")
    gid_s = gid[order]
    counts = np.bincount(gid_s, minlength=NCORES * NT * 2)
    starts = np.zeros(NCORES * NT * 2 + 1, np.int64)
    np.cumsum(counts, out=starts[1:])
    pos = np.empty(E, np.int64)
    pos[order] = np.arange(E) - starts[gid_s]
    assert pos.max() < CAP

    slot = h * CAP + pos                      # slot within node tile [0, 1024)
    t = slot // P
    p = slot % P

    # selector meta, partition-major [NC][p][nt][t][col], col = b*M + m
    c_eb = (ew[:, None] * bw[et]).astype(ml_dtypes.bfloat16)   # (E, B)
    s4 = np.zeros((NCORES, P, NT, TPT, B * M), ml_dtypes.bfloat16)
    for b in range(B):
        s4[core, p, ntl, t, b * M + m] = c_eb[:, b]
    s4 = s4.reshape(NCORES, P, NT * TPT * B * M)

    # gather indices, wrapped: [NC][p][bat][h][j//16], j = tt*128 + p_slot
    NG = NT // 4
    NB = NG // GC
    NIDX = GC * 2048
    g = ntl // 4
    k = ntl % 4
    bat = g // GC
    gl = g % GC
    th = pos // P                              # 0..3 within parity half
    tt = gl * 16 + k * 4 + th
    j = tt * P + (pos % P)
    idxv = (src >> 1).astype(np.int16)
    idx_flat = np.zeros((NCORES, NB, 2, NIDX), np.int16)
    idx_flat[core, bat, h, j] = idxv
    # Trailing pad slots of each 1024-idx call -> -1: the gather ucode skips
    # trailing negative indices, shortening the call's service time.  Keep
    # the first 4 batches unmasked (first use of the rotating xg buffers --
    # skipped slots would expose stale pre-kernel SBUF, and NaN*0 != 0 in the
    # matmul), and keep slot 0 of fully-empty calls.
    occ = np.zeros((NCORES, NB, 2, NIDX), bool)
    occ[core, bat, h, j] = True
    oc = occ.reshape(NCORES, NB, 2, NIDX // 1024, 1024)
    trail = np.flip(np.maximum.accumulate(np.flip(oc, axis=-1), axis=-1),
                    axis=-1) == 0
    trail[:, :4] = False
    trail[..., 0] = False
    ifl = idx_flat.reshape(NCORES, NB, 2, NIDX // 1024, 1024)
    ifl[trail] = -1
    idx_flat = ifl.reshape(NCORES, NB, 2, NIDX)
    # wrap16: j lives at [row j%16, col j//16], rows replicated x8
    iw = idx_flat.reshape(NCORES, NB, 2, NIDX // 16, 16)
    iw = np.swapaxes(iw, -1, -2)               # [..., 16, NIDX//16]
    iw = np.tile(iw, (1, 1, 1, 8, 1))          # [..., 128, NIDX//16]
    idx16 = np.ascontiguousarray(
        np.moveaxis(iw, 3, 1)                  # [NC, 128, NB, 2, NIDX//16]
    ).reshape(NCORES, P, NB * 2 * (NIDX // 16))

    # x tables (host cast + parity split)
    xb = x.astype(ml_dtypes.bfloat16)
    xe = np.ascontiguousarray(xb[0::2])
    xo = np.ascontiguousarray(xb[1::2])

    basesT = np.ascontiguousarray(
        bs.transpose(1, 0, 2).reshape(D, B * D)
    ).astype(ml_dtypes.bfloat16)

    # output permutation: node n (local) -> scratch row ntl*32 + (n - base)
    nodes = np.arange(N)
    rowmap = (tile_of[nodes].astype(np.int64) * M
              + nodes - base_of[nodes]).reshape(NCORES, NPC)
    return xe, xo, basesT, s4, idx16, rowmap, NT


def kernel(x, source, target, edge_type, edge_weights, base_weights, bases):
    global LAST_PROFILE
    x = np.ascontiguousarray(np.asarray(x), dtype=np.float32)
    src = np.asarray(source).astype(np.int64)
    tgt = np.asarray(target).astype(np.int64)
    et = np.asarray(edge_type).astype(np.int64)
    ew = np.ascontiguousarray(np.asarray(edge_weights), dtype=np.float32)
    bw = np.ascontiguousarray(np.asarray(base_weights), dtype=np.float32)
    bs = np.ascontiguousarray(np.asarray(bases), dtype=np.float32)

    N = x.shape[0]
    NPC = N // NCORES

    xe, xo, basesT, s4, idx16, rowmap, NT = _host_prep(x, src, tgt, et, ew, bw, bs)

    key = (N, NT)
    if key not in _PROG_CACHE:
        _PROG_CACHE[key] = _build_program(*key)
    nc = _PROG_CACHE[key]

    in_maps = [
        dict(xe=xe, xo=xo, basesT=basesT, s4=s4[c], idx16=idx16[c])
        for c in range(NCORES)
    ]
    res = run_bass_kernel_spmd(nc, in_maps, list(range(NCORES)), trace=TRACE)
    LAST_PROFILE = res
    out = np.empty((N, D), np.float32)
    for c in range(NCORES):
        scratch = res.results[c]["out"]
        out[c * NPC:(c + 1) * NPC] = scratch[rowmap[c]]
    return out
